# revision 1
# baseline (speedup 1.0000x reference)
"""GQA attention (B=2, S=2048, D=4096, 32 q-heads, 8 kv-heads) on 8 trn2
NeuronCores.

Strategy (tensor-parallel attention + token-parallel output projection):
  - core c gets wq[:, 512c:512(c+1)] (4 q-heads), wk/wv[:, 128c:128(c+1)]
    (1 kv-head), full x and full wo.
  - per core: PE-transpose x into x^T tiles (channels on partitions),
    project Q^T/K^T (head dim on partitions) and V, run attention for its
    4 heads over all tokens in the scores-transposed layout
    (S^T[k,q] tiles; softmax denominator via a ones-column matmul on the
    PE; no row-max subtraction — |scores| < ~10 so exp is safe in fp32),
  - one AllToAll flips head-sharding into token-sharding, then each core
    computes out[tokens_c, :] = attn^T.T @ wo with no cross-core
    reduction. Host concatenates the 8 token slices.
  All matmuls run in float32r (TF32-like: 1+8+11 bits, full PE rate).
"""
import numpy as np

import concourse.bass as bass
import concourse.mybir as mybir
import concourse.tile as tile
from concourse.bass_utils import run_bass_kernel_spmd

F32 = mybir.dt.float32
F32R = mybir.dt.float32r
BF16 = mybir.dt.bfloat16
AF = mybir.ActivationFunctionType
OP = mybir.AluOpType

P = 128
B, S, D = 2, 2048, 4096
NH, NKV, HD = 32, 8, 128
NCORES = 8
QH = NH // NCORES            # 4 q-heads per core
DQ = QH * HD                 # 512
TOK = B * S                  # 4096
TSLICE = TOK // NCORES       # 512 tokens per core for the wo phase
CT = D // P                  # 32 channel tiles
TCH = 256                    # phase-1 token chunk
NCH = S // TCH               # 8 chunks per batch
KTB = S // P                 # 16 key tiles per batch
QC = 512                     # attention query chunk
NQC = S // QC                # 4 per batch
SCALE = 1.0 / float(np.sqrt(HD))

# ---------------------------------------------------------------------------
# workarounds for this walrus build (max ~1 sync wait per instruction)
# ---------------------------------------------------------------------------

def _patched_drain_and_barrier(self, tick_clock, wait_clock):
    from concourse.vector_clock import ScopedClock

    nop_inst = self.nc.sync.nop(nofuse=True, hint="drain_waits")
    wait_clock.add_sem_waits(
        nop_inst.ins, ScopedClock({None: tick_clock.global_clock})
    )
    si = nop_inst.ins.sync_info
    waits = list(si.on_wait or [])
    if len(waits) > 1:
        si.on_wait = waits[:1]
        for i in range(1, len(waits)):
            extra = self.nc.sync.nop(nofuse=True, hint="drain_waits")
            extra.ins.sync_info = mybir.SyncInfo(on_wait=[waits[i]], on_update=[])
    self.nc.sync.drain()
    self.nc.all_engine_barrier()
    assert self.sems is not None
    popped = self.nc._tile_sem_poison_stack.pop()
    assert popped is self._sem_poison
    self.nc.clear_and_free_semaphores(list(self.sems.allocated().values()))
    self.nc.all_engine_barrier()


def _install_tile_patch():
    tile.TileContext._drain_and_barrier = _patched_drain_and_barrier


def _legalize_waits(nc, max_waits=1):
    n_split = 0
    for bb in nc.main_func.blocks:
        insts = bb.instructions
        new_list = []
        changed = False
        for inst in insts:
            si = inst.sync_info
            waits = list(si.on_wait) if si and si.on_wait else []
            if len(waits) > max_waits:
                keep = waits[-max_waits:]
                extra = waits[: len(waits) - max_waits]
                for i in range(0, len(extra), max_waits):
                    chunk = extra[i : i + max_waits]
                    nop = mybir.InstNoOp(
                        name=nc.get_next_instruction_name(),
                        engine=inst.engine,
                        sync_info=mybir.SyncInfo(on_wait=chunk, on_update=[]),
                        text_hint="wait_split",
                        bass_nofuse=True,
                    )
                    nc.register_instruction(nop)
                    new_list.append(nop)
                inst.sync_info = mybir.SyncInfo(
                    on_wait=keep, on_update=list(si.on_update or [])
                )
                n_split += 1
                changed = True
            new_list.append(inst)
        if changed:
            bb.instructions = new_list
    return n_split


# ---------------------------------------------------------------------------
# host-side fp32r rounding (1+8+11-bit, round to nearest even)
# ---------------------------------------------------------------------------

def _round_f32r(a):
    bits = np.ascontiguousarray(a, dtype=np.float32).view(np.uint32).astype(np.uint64)
    lsb = (bits >> 12) & 1
    bits = (((bits + 2047 + lsb) >> 12) << 12) & 0xFFFFFFFF
    return bits.astype(np.uint32).view(np.float32)


# ---------------------------------------------------------------------------
# kernel build
# ---------------------------------------------------------------------------

def _build_nc(reps=1):
    nc = bass.Bass()
    x = nc.declare_dram_parameter("x", [TOK, D], F32R, isOutput=False)
    wq = nc.declare_dram_parameter("wq", [D, DQ], F32R, isOutput=False)
    wk = nc.declare_dram_parameter("wk", [D, HD], F32R, isOutput=False)
    wv = nc.declare_dram_parameter("wv", [D, HD], F32R, isOutput=False)
    wo = nc.declare_dram_parameter("wo", [D, D], F32R, isOutput=False)
    ident = nc.declare_dram_parameter("ident", [P, P], F32R, isOutput=False)
    out = nc.declare_dram_parameter("out", [TSLICE, D], F32, isOutput=True)

    x3 = x.rearrange("(tt p) d -> tt p d", p=P)          # [32, 128, 4096]
    wq3 = wq.rearrange("(ct p) m -> p ct m", p=P)        # [128, 32, 512]
    wk3 = wk.rearrange("(ct p) m -> p ct m", p=P)        # [128, 32, 128]
    wv3 = wv.rearrange("(ct p) m -> p ct m", p=P)        # [128, 32, 128]
    wo3 = wo.rearrange("(ht p) e -> p ht e", p=P)        # [128, 32, 4096]
    out3 = out.rearrange("(tt p) e -> p tt e", p=P)      # [128, 4, 4096]

    with tile.TileContext(nc) as tc:
        with (
            tc.tile_pool(name="consts", bufs=1) as consts,
            tc.tile_pool(name="dram", bufs=1, space="DRAM") as dram,
        ):
            identity = consts.tile([P, P], F32R)
            nc.sync.dma_start(identity[:], ident[:])
            ones_f = consts.tile([P, 1], F32)
            nc.gpsimd.memset(ones_f[:], 1.0)
            ones_col = consts.tile([P, 1], F32R)
            nc.vector.tensor_copy(ones_col[:], ones_f[:])
            ones_rf = consts.tile([1, P], F32)
            nc.gpsimd.memset(ones_rf[:], 1.0)
            ones_row = consts.tile([1, P], F32R)
            nc.vector.tensor_copy(ones_row[:], ones_rf[:])

            a2a_in_lo = dram.tile([NCORES, 2 * HD, TSLICE], F32R)
            a2a_in_hi = dram.tile([NCORES, 2 * HD, TSLICE], F32R)
            a2a_out_lo = dram.tile([NCORES, 2 * HD, TSLICE], F32R)
            a2a_out_hi = dram.tile([NCORES, 2 * HD, TSLICE], F32R)

            for rep in range(reps):
              with (
                  tc.tile_pool(name="wts", bufs=1) as wts,
                  tc.tile_pool(name="batch", bufs=1) as batch,
                  tc.tile_pool(name="xs", bufs=2) as xsp,
                  tc.tile_pool(name="xts", bufs=1) as xtsp,
                  tc.tile_pool(name="expp", bufs=2) as expp,
                  tc.tile_pool(name="aop", bufs=1) as aop,
                  tc.tile_pool(name="recp", bufs=1) as recp,
                  tc.tile_pool(name="qnp", bufs=2) as qnp,
                  tc.tile_pool(name="psA", bufs=2, space="PSUM") as psA,
                  tc.tile_pool(name="psB", bufs=1, space="PSUM") as psB,
              ):
                wq_sb = wts.tile([P, CT, DQ], F32R)
                nc.scalar.dma_start(wq_sb[:], wq3[:])
                wkv_sb = wts.tile([P, CT, 2 * HD], F32R)
                nc.scalar.dma_start(wkv_sb[:, :, 0:HD], wk3[:])
                nc.scalar.dma_start(wkv_sb[:, :, HD:2 * HD], wv3[:])
                for b in range(B):
                    qt_sb = batch.tile([P, QH, S], F32R, tag="qt")
                    kt_sb = batch.tile([P, S], F32R, tag="kt")
                    kv_sb = batch.tile([P, KTB, 2 * HD], F32R, tag="kv")

                    # ---- phase 1: transpose x chunk + QKV projections ----
                    for ch in range(NCH):
                        xts_t = xtsp.tile([P, CT, TCH], F32R, tag="xts")
                        for i in range(2):  # two 128-token tiles per chunk
                            tt = b * (S // P) + ch * 2 + i
                            for eighth in range(8):
                                xst = xsp.tile([P, D // 8], F32R, tag="xs")
                                nc.sync.dma_start(
                                    xst[:], x3[tt, :, eighth * (D // 8):(eighth + 1) * (D // 8)]
                                )
                                pst = psA.tile([P, 4, P], F32R, tag="big")
                                for u in range(4):
                                    nc.tensor.matmul(
                                        pst[:, u, :],
                                        xst[:, u * P:(u + 1) * P],
                                        identity[:],
                                        is_transpose=True,
                                        skip_group_check=(u > 0),
                                    )
                                ct0 = eighth * 4
                                nc.vector.tensor_copy(
                                    xts_t[:, ct0:ct0 + 4, i * P:(i + 1) * P], pst[:]
                                )
                        for tsub in range(2):
                            kt_idx = ch * 2 + tsub
                            xsl = slice(tsub * P, (tsub + 1) * P)
                            psq = psA.tile([P, DQ], F32, tag="med")
                            pskv = psA.tile([P, 2 * HD], F32, tag="med")
                            for ct in range(CT):
                                nc.tensor.matmul(
                                    psq[:], xts_t[:, ct, xsl], wq_sb[:, ct, :],
                                    start=(ct == 0), stop=(ct == CT - 1),
                                )
                                nc.tensor.matmul(
                                    pskv[:], xts_t[:, ct, xsl], wkv_sb[:, ct, :],
                                    start=(ct == 0), stop=(ct == CT - 1),
                                )
                            qn = qnp.tile([P, DQ], F32R, tag="qn")
                            nc.vector.tensor_copy(qn[:], psq[:])
                            nc.vector.tensor_copy(kv_sb[:, kt_idx, :], pskv[:])
                            pst = psA.tile([P, 4, P], F32R, tag="big")
                            for hd in range(QH):
                                nc.tensor.matmul(
                                    pst[:, hd, :],
                                    qn[:, hd * P:(hd + 1) * P],
                                    identity[:],
                                    is_transpose=True,
                                    skip_group_check=(hd > 0),
                                )
                            nc.vector.tensor_copy(
                                qt_sb[:, 0:QH, kt_idx * P:(kt_idx + 1) * P], pst[:]
                            )
                            pskt = psB.tile([P, HD], F32R, tag="sm")
                            nc.tensor.matmul(
                                pskt[:], kv_sb[:, kt_idx, 0:HD], identity[:],
                                is_transpose=True,
                            )
                            nc.vector.tensor_copy(
                                kt_sb[:, kt_idx * P:(kt_idx + 1) * P], pskt[:]
                            )

                    # ---- phase 2: attention for this batch ----
                    for qc in range(NQC):
                        j = b * NQC + qc     # destination core for these tokens
                        qsl = slice(qc * QC, (qc + 1) * QC)
                        for h in range(QH):
                            numer = psA.tile([P, QC], F32, tag="med")
                            den4 = psB.tile([P, QC], F32, tag="dn")
                            for kp in range(KTB // 2):
                                pss = psA.tile([P, 2, QC], F32, tag="big")
                                for u in range(2):
                                    kt = kp * 2 + u
                                    nc.tensor.matmul(
                                        pss[:, u, :],
                                        kt_sb[:, kt * P:(kt + 1) * P],
                                        qt_sb[:, h, qsl],
                                        start=True, stop=True,
                                        skip_group_check=(u > 0),
                                    )
                                et = expp.tile([P, 2, QC], F32R, tag="exp")
                                nc.scalar.activation(et[:], pss[:], AF.Exp, scale=SCALE)
                                for u in range(2):
                                    kt = kp * 2 + u
                                    first = kp == 0 and u == 0
                                    last = kp == KTB // 2 - 1 and u == 1
                                    nc.tensor.matmul(
                                        numer[:], kv_sb[:, kt, HD:2 * HD], et[:, u, :],
                                        start=first, stop=last,
                                    )
                                    nc.tensor.matmul(
                                        den4[0:1, :], ones_col[:], et[:, u, :],
                                        start=first, stop=last,
                                    )
                            rec = recp.tile([1, QC], F32R, tag="rec")
                            with nc.allow_low_precision(reason="softmax recip in f32r"):
                                nc.vector.reciprocal(rec[:], den4[0:1, :])
                            rbc = psA.tile([P, QC], F32, tag="big")
                            nc.tensor.matmul(
                                rbc[:], ones_row[:], rec[:], start=True, stop=True
                            )
                            rbs = qnp.tile([P, QC], F32R, tag="qn")
                            nc.vector.tensor_copy(rbs[:], rbc[:])
                            ao = aop.tile([P, QC], F32R, tag="ao")
                            nc.vector.tensor_tensor(
                                ao[:], numer[:], rbs[:], OP.mult
                            )
                            a2a_dst = a2a_in_lo if h < 2 else a2a_in_hi
                            nc.sync.dma_start(
                                a2a_dst[j, (h % 2) * P:(h % 2 + 1) * P, :], ao[:]
                            )

              # ---- AllToAll: head-sharded -> token-sharded (two waves) ----
              if True:
                nc.gpsimd.collective_compute(
                    "AllToAll",
                    OP.bypass,
                    ins=[a2a_in_lo.opt()],
                    outs=[a2a_out_lo.opt()],
                    replica_groups=[list(range(NCORES))],
                )
                nc.gpsimd.collective_compute(
                    "AllToAll",
                    OP.bypass,
                    ins=[a2a_in_hi.opt()],
                    outs=[a2a_out_hi.opt()],
                    replica_groups=[list(range(NCORES))],
                )

                # ---- phase 3: out[tokens_c, :] = attnT.T @ wo ----
                with (
                    tc.tile_pool(name="wop", bufs=1) as wop,
                    tc.tile_pool(name="wos", bufs=64) as wos,
                    tc.tile_pool(name="outp", bufs=3) as outp,
                    tc.tile_pool(name="ps3", bufs=2, space="PSUM") as ps3,
                ):
                    a2a_sb_lo = wop.tile([P, 16, TSLICE], F32R)
                    nc.sync.dma_start(
                        a2a_sb_lo[:],
                        a2a_out_lo[:].rearrange("j (g2 p) t -> p (j g2) t", p=P),
                    )
                    a2a_sb_hi = wop.tile([P, 16, TSLICE], F32R)
                    nc.sync.dma_start(
                        a2a_sb_hi[:],
                        a2a_out_hi[:].rearrange("j (g2 p) t -> p (j g2) t", p=P),
                    )
                    psos_live = {}

                    def group_a(ec):
                        esl = slice(ec * 512, (ec + 1) * 512)
                        psos_live[ec] = [
                            ps3.tile([P, 512], F32, tag=f"wo{tt}", name=f"pso{ec}_{tt}")
                            for tt in range(TSLICE // P)
                        ]
                        first = True
                        for j in range(NCORES):
                            for hl in range(2):
                                ht = 4 * j + hl
                                wo_t = wos.tile([P, 512], F32R, tag="wo_t", name="wo_a")
                                nc.scalar.dma_start(wo_t[:], wo3[:, ht, esl])
                                for tt in range(TSLICE // P):
                                    nc.tensor.matmul(
                                        psos_live[ec][tt][:],
                                        a2a_sb_lo[:, j * 2 + hl, tt * P:(tt + 1) * P],
                                        wo_t[:],
                                        start=first, stop=False,
                                    )
                                first = False

                    def group_b(ec):
                        esl = slice(ec * 512, (ec + 1) * 512)
                        for j in range(NCORES):
                            for hl in range(2):
                                ht = 4 * j + 2 + hl
                                wo_t = wos.tile([P, 512], F32R, tag="wo_t", name="wo_b")
                                nc.scalar.dma_start(wo_t[:], wo3[:, ht, esl])
                                for tt in range(TSLICE // P):
                                    nc.tensor.matmul(
                                        psos_live[ec][tt][:],
                                        a2a_sb_hi[:, j * 2 + hl, tt * P:(tt + 1) * P],
                                        wo_t[:],
                                        start=False,
                                        stop=(j == NCORES - 1 and hl == 1),
                                    )
                        for tt in range(TSLICE // P):
                            ot = outp.tile([P, 512], F32, tag="ot", name="ot")
                            nc.vector.tensor_copy(ot[:], psos_live[ec][tt][:])
                            nc.sync.dma_start(out3[:, tt, esl], ot[:])
                        del psos_live[ec]

                    group_a(0)
                    for ec in range(1, 8):
                        group_a(ec)
                        group_b(ec - 1)
                    group_b(7)

    _legalize_waits(nc)
    return nc


_NC_CACHE = {}


def _get_nc(reps=1):
    if reps not in _NC_CACHE:
        _install_tile_patch()
        _NC_CACHE[reps] = _build_nc(reps)
    return _NC_CACHE[reps]


def make_in_maps(x, wq, wk, wv, wo):
    xf = _round_f32r(np.asarray(x, dtype=np.float32).reshape(TOK, D))
    wqf = _round_f32r(wq)
    wkf = _round_f32r(wk)
    wvf = _round_f32r(wv)
    wof = _round_f32r(wo)
    identv = np.eye(P, dtype=np.float32)
    in_maps = []
    for c in range(NCORES):
        in_maps.append({
            "x": xf,
            "ident": identv,
            "wq": np.ascontiguousarray(wqf[:, c * DQ:(c + 1) * DQ]),
            "wk": np.ascontiguousarray(wkf[:, c * HD:(c + 1) * HD]),
            "wv": np.ascontiguousarray(wvf[:, c * HD:(c + 1) * HD]),
            "wo": wof,
        })
    return in_maps


def assemble_output(results):
    out = np.concatenate([results[c]["out"] for c in range(NCORES)], axis=0)
    return out.reshape(B, S, D)


def kernel(x, wq, wk, wv, wo):
    nc = _get_nc(reps=1)
    in_maps = make_in_maps(x, wq, wk, wv, wo)
    res = run_bass_kernel_spmd(nc, in_maps, list(range(NCORES)))
    return assemble_output(res.results)


if __name__ == "__main__":
    rng = np.random.default_rng(0)
    xv = rng.standard_normal((B, S, D), dtype=np.float32)
    wqv = (rng.standard_normal((D, NH * HD), dtype=np.float32) * 0.02)
    wkv = (rng.standard_normal((D, NKV * HD), dtype=np.float32) * 0.02)
    wvv = (rng.standard_normal((D, NKV * HD), dtype=np.float32) * 0.02)
    wov = (rng.standard_normal((NH * HD, D), dtype=np.float32) * 0.02)
    got = kernel(xv, wqv, wkv, wvv, wov)
    print("kernel output", got.shape, got.dtype)



# revision 12
# speedup vs baseline: 52.1713x; 52.1713x over previous
"""GQA attention (B=2, S=2048, D=4096, 32 q-heads, 8 kv-heads) on 8 trn2
NeuronCores — fp16 tensor-parallel version.

Per core c (4 q-heads, 1 kv-head):
  - x^T tiles produced by the XBAR DMA-transpose engine straight from
    DRAM (fp16), no PE/DVE transpose cost.
  - Q^T projected directly (weights stationary, x^T moving), K/V fused
    into one 256-wide moving matmul; K transposed on the PE.
  - attention in the scores-transposed layout; exp on ACT with a
    folded -ln(16) bias (keeps fp16 den < 65504/16); softmax denominator
    accumulated by an in-register DVE halving tree + one ones-matmul.
  - 4 AllToAll waves (one per 1024 global tokens) flip head-sharding to
    token-sharding as soon as each wave's attention completes; the output
    projection runs in two halves, each sharing one streamed pass over
    wo for two waves, overlapping the remaining compute.
Output: core j returns out rows [128w + i] = global token 1024w + 128j + i.
"""
import numpy as np

import concourse.bass as bass
import concourse.mybir as mybir
import concourse.tile as tile
from concourse.bass_utils import run_bass_kernel_spmd

F16 = mybir.dt.float16
F32 = mybir.dt.float32
AF = mybir.ActivationFunctionType
OP = mybir.AluOpType

P = 128
B, S, D = 2, 2048, 4096
NH, NKV, HD = 32, 8, 128
NCORES = 8
QH = NH // NCORES            # 4 q-heads per core
DQ = QH * HD                 # 512
TOK = B * S                  # 4096
TSLICE = TOK // NCORES       # 512 output rows per core
CT = D // P                  # 32 channel tiles
CHUNK = 256                  # phase-1 token chunk
NCH = S // CHUNK             # 8 chunks per batch
KTB = S // P                 # 16 key tiles per batch
QC = 512                     # attention query chunk
NQC = S // QC                # 4 per batch
NWAVE = 4                    # a2a waves; wave w = tokens [1024w, 1024w+1024)
ECW = 256                    # phase-3 output-column chunk
SCALE = 1.0 / float(np.sqrt(HD))
EBIAS = -float(np.log(16.0))

# ---------------------------------------------------------------------------
# workarounds for this walrus build (max ~1 sync wait per instruction)
# ---------------------------------------------------------------------------

def _patched_drain_and_barrier(self, tick_clock, wait_clock):
    from concourse.vector_clock import ScopedClock

    nop_inst = self.nc.sync.nop(nofuse=True, hint="drain_waits")
    wait_clock.add_sem_waits(
        nop_inst.ins, ScopedClock({None: tick_clock.global_clock})
    )
    si = nop_inst.ins.sync_info
    waits = list(si.on_wait or [])
    if len(waits) > 1:
        si.on_wait = waits[:1]
        for i in range(1, len(waits)):
            extra = self.nc.sync.nop(nofuse=True, hint="drain_waits")
            extra.ins.sync_info = mybir.SyncInfo(on_wait=[waits[i]], on_update=[])
    self.nc.sync.drain()
    self.nc.all_engine_barrier()
    assert self.sems is not None
    popped = self.nc._tile_sem_poison_stack.pop()
    assert popped is self._sem_poison
    self.nc.clear_and_free_semaphores(list(self.sems.allocated().values()))
    self.nc.all_engine_barrier()


def _install_tile_patch():
    tile.TileContext._drain_and_barrier = _patched_drain_and_barrier


def _legalize_waits(nc, max_waits=1):
    n_split = 0
    for bb in nc.main_func.blocks:
        insts = bb.instructions
        new_list = []
        changed = False
        for inst in insts:
            si = inst.sync_info
            waits = list(si.on_wait) if si and si.on_wait else []
            if len(waits) > max_waits:
                keep = waits[-max_waits:]
                extra = waits[: len(waits) - max_waits]
                for i in range(0, len(extra), max_waits):
                    chunk = extra[i : i + max_waits]
                    nop = mybir.InstNoOp(
                        name=nc.get_next_instruction_name(),
                        engine=inst.engine,
                        sync_info=mybir.SyncInfo(on_wait=chunk, on_update=[]),
                        text_hint="wait_split",
                        bass_nofuse=True,
                    )
                    nc.register_instruction(nop)
                    new_list.append(nop)
                inst.sync_info = mybir.SyncInfo(
                    on_wait=keep, on_update=list(si.on_update or [])
                )
                n_split += 1
                changed = True
            new_list.append(inst)
        if changed:
            bb.instructions = new_list
    return n_split


# ---------------------------------------------------------------------------
# kernel build
# ---------------------------------------------------------------------------

def _build_nc(reps=1):
    nc = bass.Bass()
    x = nc.declare_dram_parameter("x", [TOK, D], F16, isOutput=False)
    wq = nc.declare_dram_parameter("wq", [D, DQ], F16, isOutput=False)
    wk = nc.declare_dram_parameter("wk", [D, HD], F16, isOutput=False)
    wv = nc.declare_dram_parameter("wv", [D, HD], F16, isOutput=False)
    wo = nc.declare_dram_parameter("wo", [D, D], F16, isOutput=False)
    out = nc.declare_dram_parameter("out", [TSLICE, D], F32, isOutput=True)

    wq3 = wq.rearrange("(ct p) m -> p ct m", p=P)        # [128, 32, 512]
    wk3 = wk.rearrange("(ct p) m -> p ct m", p=P)        # [128, 32, 128]
    wv3 = wv.rearrange("(ct p) m -> p ct m", p=P)        # [128, 32, 128]
    wo3 = wo.rearrange("(ht p) e -> p ht e", p=P)        # [128, 32, 4096]
    out3 = out.rearrange("(tt p) e -> p tt e", p=P)      # [128, 4, 4096]

    from contextlib import ExitStack

    with tile.TileContext(nc) as tc:
        with ExitStack() as stack:
            pools = {}
            for nm, kw in [
                ("consts", dict(bufs=1)),
                ("dram", dict(bufs=1, space="DRAM")),
                ("wts", dict(bufs=1)),
                ("batch", dict(bufs=1)),
                ("xts", dict(bufs=2)),
                ("etp", dict(bufs=2)),
                ("esp", dict(bufs=2)),
                ("recp", dict(bufs=2)),
                ("rbsp", dict(bufs=2)),
                ("aop", dict(bufs=2)),
                ("p3sb", dict(bufs=2)),
                ("wos", dict(bufs=8)),
                ("outp", dict(bufs=2)),
                ("p1ps", dict(bufs=1, space="PSUM")),
                ("pssp", dict(bufs=2, space="PSUM")),
                ("nump", dict(bufs=3, space="PSUM")),
                ("ps3p", dict(bufs=1, space="PSUM")),
            ]:
                pools[nm] = stack.enter_context(tc.tile_pool(name=nm, **kw))
            consts, dram, wts, batch = (
                pools["consts"], pools["dram"], pools["wts"], pools["batch"]
            )
            xtsp, etp, esp, recp = (
                pools["xts"], pools["etp"], pools["esp"], pools["recp"]
            )
            rbsp, aop, p3sb, wos, outp = (
                pools["rbsp"], pools["aop"], pools["p3sb"], pools["wos"],
                pools["outp"],
            )
            p1ps = pools["p1ps"]
            pssp, nump, ps3p = pools["pssp"], pools["nump"], pools["ps3p"]
            dnp = nump
            ones_f = consts.tile([P, 1], F32)
            nc.gpsimd.memset(ones_f[:], 1.0)
            ones_col = consts.tile([P, 1], F16)
            nc.vector.tensor_copy(ones_col[:], ones_f[:])
            ones_rf = consts.tile([1, P], F32)
            nc.gpsimd.memset(ones_rf[:], 1.0)
            ones_row = consts.tile([1, P], F16)
            nc.vector.tensor_copy(ones_row[:], ones_rf[:])
            ebias = consts.tile([P, 1], F32)
            nc.gpsimd.memset(ebias[:], EBIAS)

            a2a_in = [dram.tile([NCORES, DQ, P], F16, name=f"a2ai{w}")
                      for w in range(NWAVE)]
            a2a_out = [dram.tile([NCORES, DQ, P], F16, name=f"a2ao{w}")
                       for w in range(NWAVE)]

            state = {}

            def load_weights():
                wq_sb = wts.tile([P, CT, DQ], F16, tag="wq")
                nc.scalar.dma_start(wq_sb[:], wq3[:])
                wkv_sb = wts.tile([P, CT, 2 * HD], F16, tag="wkv")
                nc.scalar.dma_start(wkv_sb[:, :, 0:HD], wk3[:])
                nc.scalar.dma_start(wkv_sb[:, :, HD:2 * HD], wv3[:])
                state["wq"] = wq_sb
                state["wkv"] = wkv_sb
                qt_sb, kt_sb, kv_sb = {}, {}, {}
                for b in range(B):
                    qt_sb[b] = batch.tile([P, QH, S], F16, tag=f"qt{b}", name=f"qt{b}")
                    kt_sb[b] = batch.tile([P, S], F16, tag=f"kt{b}", name=f"kt{b}")
                    kv_sb[b] = batch.tile(
                        [P, KTB, 2 * HD], F16, tag=f"kv{b}", name=f"kv{b}"
                    )
                state["qt"], state["kt"], state["kv"] = qt_sb, kt_sb, kv_sb

            def phase1(b):
                wq_sb, wkv_sb = state["wq"], state["wkv"]
                qt_sb, kt_sb, kv_sb = state["qt"], state["kt"], state["kv"]
                for ch in range(NCH):
                    xts = xtsp.tile([P, CT, CHUNK], F16, tag="xts")
                    tok0 = b * S + ch * CHUNK
                    for ct in range(CT):
                        nc.sync.dma_start_transpose(
                            xts[:, ct, :],
                            x[tok0:tok0 + CHUNK, ct * P:(ct + 1) * P],
                        )
                    for d in range(QH):
                        p1 = p1ps.tile([P, 2, CHUNK], F32, tag="p1")
                        psq = p1[:, 0, :]
                        for ct in range(CT):
                            nc.tensor.matmul(
                                psq, wq_sb[:, ct, d * P:(d + 1) * P],
                                xts[:, ct, :],
                                start=(ct == 0), stop=(ct == CT - 1),
                            )
                        nc.vector.tensor_copy(
                            qt_sb[b][:, d, ch * CHUNK:(ch + 1) * CHUNK], psq
                        )
                        if d >= 2:
                            tsub = d - 2
                            kt_idx = ch * 2 + tsub
                            pkv = p1[:, 1, :]
                            for ct in range(CT):
                                nc.tensor.matmul(
                                    pkv,
                                    xts[:, ct, tsub * P:(tsub + 1) * P],
                                    wkv_sb[:, ct, :],
                                    start=(ct == 0), stop=(ct == CT - 1),
                                )
                            nc.vector.tensor_copy(kv_sb[b][:, kt_idx, :], pkv)
                            nc.sync.dma_start_transpose(
                                kt_sb[b][:, kt_idx * P:(kt_idx + 1) * P],
                                kv_sb[b][:, kt_idx, 0:HD],
                            )

            def phase2(b):
                qt_sb, kt_sb, kv_sb = state["qt"], state["kt"], state["kv"]
                for qc in range(NQC):
                    qsl = slice(qc * QC, (qc + 1) * QC)
                    w = 2 * b + qc // 2
                    for h in range(QH):
                        et = etp.tile([P, KTB, QC], F16, tag="et")
                        numer = nump.tile([P, QC], F32, tag="num")
                        for kt in range(KTB):
                            pss = pssp.tile([P, QC], F32, tag="pss")
                            nc.tensor.matmul(
                                pss[:], kt_sb[b][:, kt * P:(kt + 1) * P],
                                qt_sb[b][:, h, qsl], start=True, stop=True,
                            )
                            nc.scalar.activation(
                                et[:, kt, :], pss[:], AF.Exp,
                                bias=ebias[:], scale=SCALE,
                            )
                            nc.tensor.matmul(
                                numer[:], kv_sb[b][:, kt, HD:2 * HD],
                                et[:, kt, :],
                                start=(kt == 0), stop=(kt == KTB - 1),
                            )
                        # in-place halving tree: et[:, 0, :] + et[:, 1, :]
                        # ends up holding per-lane partial sums
                        nc.vector.tensor_tensor(
                            et[:, 0:8, :], et[:, 0:8, :], et[:, 8:16, :], OP.add
                        )
                        nc.vector.tensor_tensor(
                            et[:, 0:4, :], et[:, 0:4, :], et[:, 4:8, :], OP.add
                        )
                        nc.vector.tensor_tensor(
                            et[:, 0:2, :], et[:, 0:2, :], et[:, 2:4, :], OP.add
                        )
                        esum = esp.tile([P, QC], F16, tag="esum")
                        nc.vector.tensor_tensor(
                            esum[:], et[:, 0, :], et[:, 1, :], OP.add
                        )
                        den = dnp.tile([P, QC], F32, tag="num", name="den")
                        nc.tensor.matmul(
                            den[0:1, :], ones_col[:], esum[:],
                            start=True, stop=True,
                        )
                        rec = recp.tile([1, QC], F16, tag="rec")
                        with nc.allow_low_precision(reason="softmax recip fp16"):
                            nc.vector.reciprocal(rec[:], den[0:1, :])
                        rbc = dnp.tile([P, QC], F32, tag="num", name="rbc")
                        nc.tensor.matmul(
                            rbc[:], ones_row[:], rec[:], start=True, stop=True
                        )
                        rbs = rbsp.tile([P, QC], F16, tag="rbs")
                        nc.vector.tensor_copy(rbs[:], rbc[:])
                        ao = aop.tile([P, QC], F16, tag="ao")
                        nc.vector.tensor_tensor(ao[:], numer[:], rbs[:], OP.mult)
                        for m in range(4):
                            j = 4 * (qc % 2) + m
                            nc.sync.dma_start(
                                a2a_in[w][j, h * P:(h + 1) * P, :],
                                ao[:, m * P:(m + 1) * P],
                            )
                    if qc % 2 == 1:
                        nc.gpsimd.collective_compute(
                            "AllToAll",
                            OP.bypass,
                            ins=[a2a_in[w].opt()],
                            outs=[a2a_out[w].opt()],
                            replica_groups=[list(range(NCORES))],
                        )

            def phase3_half(wA, wB):
                sbs = {}
                for w in (wA, wB):
                    sb = p3sb.tile([P, NH, P], F16, tag="a2asb")
                    nc.sync.dma_start(
                        sb[:],
                        a2a_out[w][:].rearrange("j (h p) t -> p (j h) t", p=P),
                    )
                    sbs[w] = sb
                for ec in range(D // ECW):
                    esl = slice(ec * ECW, (ec + 1) * ECW)
                    ps3a = ps3p.tile([P, ECW], F32, tag="ps3a", name="ps3a")
                    ps3b = ps3p.tile([P, ECW], F32, tag="ps3b", name="ps3b")
                    for ht in range(NH):
                        wo_t = wos.tile([P, ECW], F16, tag="wo_t")
                        nc.sync.dma_start(wo_t[:], wo3[:, ht, esl])
                        nc.tensor.matmul(
                            ps3a[:], sbs[wA][:, ht, :], wo_t[:],
                            start=(ht == 0), stop=(ht == NH - 1),
                        )
                        nc.tensor.matmul(
                            ps3b[:], sbs[wB][:, ht, :], wo_t[:],
                            start=(ht == 0), stop=(ht == NH - 1),
                        )
                    for ps, w in ((ps3a, wA), (ps3b, wB)):
                        ot = outp.tile([P, ECW], F32, tag="ot")
                        nc.vector.tensor_copy(ot[:], ps[:])
                        nc.sync.dma_start(out3[:, w, esl], ot[:])

            for rep in range(reps):
                load_weights()
                phase1(0)
                phase2(0)
                phase1(1)
                phase3_half(0, 1)
                phase2(1)
                phase3_half(2, 3)

    _legalize_waits(nc)
    return nc


_NC_CACHE = {}


def _get_nc(reps=1):
    if reps not in _NC_CACHE:
        _install_tile_patch()
        _NC_CACHE[reps] = _build_nc(reps)
    return _NC_CACHE[reps]


def make_in_maps(x, wq, wk, wv, wo):
    xf = np.asarray(x, dtype=np.float32).reshape(TOK, D).astype(np.float16)
    wqf = np.asarray(wq, dtype=np.float32).astype(np.float16)
    wkf = np.asarray(wk, dtype=np.float32).astype(np.float16)
    wvf = np.asarray(wv, dtype=np.float32).astype(np.float16)
    wof = np.asarray(wo, dtype=np.float32).astype(np.float16)
    in_maps = []
    for c in range(NCORES):
        in_maps.append({
            "x": xf,
            "wq": np.ascontiguousarray(wqf[:, c * DQ:(c + 1) * DQ]),
            "wk": np.ascontiguousarray(wkf[:, c * HD:(c + 1) * HD]),
            "wv": np.ascontiguousarray(wvf[:, c * HD:(c + 1) * HD]),
            "wo": wof,
        })
    return in_maps


def assemble_output(results):
    full = np.empty((TOK, D), dtype=np.float32)
    for j in range(NCORES):
        r = results[j]["out"].reshape(TSLICE, D)
        for w in range(NWAVE):
            full[1024 * w + P * j: 1024 * w + P * (j + 1)] = r[P * w: P * (w + 1)]
    return full.reshape(B, S, D)


def kernel(x, wq, wk, wv, wo):
    nc = _get_nc(reps=1)
    in_maps = make_in_maps(x, wq, wk, wv, wo)
    res = run_bass_kernel_spmd(nc, in_maps, list(range(NCORES)))
    return assemble_output(res.results)


if __name__ == "__main__":
    rng = np.random.default_rng(0)
    xv = rng.standard_normal((B, S, D), dtype=np.float32)
    wqv = (rng.standard_normal((D, NH * HD), dtype=np.float32) * 0.02)
    wkv = (rng.standard_normal((D, NKV * HD), dtype=np.float32) * 0.02)
    wvv = (rng.standard_normal((D, NKV * HD), dtype=np.float32) * 0.02)
    wov = (rng.standard_normal((NH * HD, D), dtype=np.float32) * 0.02)
    got = kernel(xv, wqv, wkv, wvv, wov)
    print("kernel output", got.shape, got.dtype)
